# revision 24
# baseline (speedup 1.0000x reference)
"""Trainium2 Bass kernel for the CementPINN MLP (dense_mlp, 8 cores).

Data-parallel: x [32768, 8] is sharded along batch into 8 shards of 4096
rows; MLP weights are replicated on every core.  Per core the MLP runs
feature-major (activations h^T [feat, batch]); every layer is
out^T[m] = sum_k W[k,m]^T @ h^T[k] with the weight tile stationary.

L2/L3/L4 matmuls run in fp8 (e4m3) with MatmulPerfMode.DoubleRow: each
instruction contracts a PAIR of 128-feature k-tiles (stationary [128,2,128],
moving [128,2,512]) at ~1.5x the fp32r MAC rate.  Host-side the weights are
pre-scaled by powers of two (W2x4, W3x4, W4x16) so the fp8 encoding stays in
the normal range; activations carry the compounded scale (h1'=4h1, h2'=16h2,
h3'=64h3) and each ReLU stage is one fused instruction:
  ACT:  relu(psum + b')        DVE: (psum + b') max 0   (biases pre-scaled)
The raw MLP output is psum/1024 + b4.  ReLU stages split evenly over the
scalar/vector engines.  L1 (K=9: x plus a ones-row that carries the bias,
bf16) is packed 4-wide into PE row groups via tile_position and
software-pipelined two chunks ahead (groups interleaved into L2's m-loop so
their PSUM slots reuse long-freed buffers).  Weights stream over both HW
DMA queues (SP + Activation) with the first pairs split small because each
completion semaphore carries multi-us latency.  The physics-constraint
clamp is computed batch-major on [128, 32] tiles from a host-pretransposed
copy of x; the per-chunk raw row [1, 512] is converted to batch-major with
a single SBUF->SBUF strided DMA for chunks 0-6 and via PE-transpose into
PSUM for the last chunk (keeps DMA-semaphore latency out of the tail).
"""

import numpy as np

import concourse.bacc as bacc
import concourse.mybir as mybir
import concourse.tile as tile
from concourse.bass_utils import run_bass_kernel_spmd

F32 = mybir.dt.float32
F32R = mybir.dt.float32r
F8 = mybir.dt.float8e4
BF16 = mybir.dt.bfloat16
AF = mybir.ActivationFunctionType
ALU = mybir.AluOpType
DR = mybir.MatmulPerfMode.DoubleRow

N_CORES = 8
B = 32768
BC = B // N_CORES  # 4096 rows per core
D_IN = 8
H = 1024
P = 128
NB = 512  # batch columns per chunk (= one fp32 PSUM bank)
NCH = BC // NB  # 8 chunks per core
KT = H // P  # 8 feature tiles
QT = KT // 2  # 4 k-tile pairs (DoubleRow)
JT = BC // P  # 32 batch-major columns

# weight pre-scales (powers of two; folded back out via biases / raw stage)
S1 = 4.0  # W1 *= 4 (fp32r, exact)     -> h1' = 4 h1
S2 = 4.0  # W2 *= 4 (fp8)              -> h2' = 16 h2
S3 = 4.0  # W3 *= 4 (fp8)              -> h3' = 64 h3
S4 = 16.0  # W4 *= 16 (fp8)            -> psum4 = 1024 * (W4^T h3)
RAW_DIV = S1 * S2 * S3 * S4  # 1024

_CACHE = {}


def _build_nc():
    nc = bacc.Bacc("TRN2", target_bir_lowering=False, debug=False)

    NBC = KT + 2 * KT + 1  # b1 | b23 | b4
    D1 = D_IN + 1  # extra contraction row carries the L1 bias
    wx = nc.declare_dram_parameter("wx", [4 * D1, H + BC], BF16, isOutput=False)
    bconsts = nc.declare_dram_parameter("bconsts", [P, NBC], F32, isOutput=False)
    xc = nc.declare_dram_parameter("xc", [P, D_IN * JT], F32, isOutput=False)
    w2 = nc.declare_dram_parameter("w2", [P, KT, H], F8, isOutput=False)
    w3 = nc.declare_dram_parameter("w3", [P, KT, H], F8, isOutput=False)
    w4 = nc.declare_dram_parameter("w4", [P, KT, 16], F8, isOutput=False)
    out_d = nc.declare_dram_parameter("out_bm", [P, JT], F32, isOutput=True)

    with tile.TileContext(nc) as tc:
        with (
            tc.tile_pool(name="wts", bufs=1) as wp,
            tc.tile_pool(name="xin", bufs=1) as xp,
            tc.tile_pool(name="acts", bufs=3) as hp,
            tc.tile_pool(name="raw", bufs=2) as rp,
            tc.tile_pool(name="cst", bufs=1) as cp,
            tc.tile_pool(name="ps", bufs=8, space="PSUM") as pp,
        ):
            # ---- sync (HW DGE) queue: biases first (ReLUs need them
            # early), then x+W1 in bf16, host-replicated into the 4 PE row
            # groups (one small DMA per group), then one W2 pair, then xc.
            # The scalar HW queue streams the other weights in parallel.
            wx_sb = wp.tile([P, H + BC], BF16, tag="wx")
            for g in range(4):
                r0 = 32 * g
                nc.sync.dma_start(
                    wx_sb[r0 : r0 + D1, :], wx[g * D1 : (g + 1) * D1, :]
                )
            bc_sb = cp.tile([P, NBC], F32, tag="bconsts")
            nc.sync.dma_start(bc_sb[:], bconsts[:])
            w2_sb = wp.tile([P, KT, H], F8, tag="w2")
            nc.sync.dma_start(w2_sb[:, 4:6, :512], w2[:, 4:6, :512])
            nc.sync.dma_start(w2_sb[:, 4:6, 512:], w2[:, 4:6, 512:])
            nc.sync.dma_start(w2_sb[:, 6:8, :512], w2[:, 6:8, :512])
            nc.sync.dma_start(w2_sb[:, 6:8, 512:], w2[:, 6:8, 512:])
            xc_sb = cp.tile([P, D_IN * JT], F32, tag="xc")
            nc.sync.dma_start(xc_sb[:], xc[:])
            w1_sb = wx_sb[:, :H]
            b1_sb = bc_sb[:, :KT]
            b23_sb = bc_sb[:, KT : 3 * KT]
            b4_sb = bc_sb[:, 3 * KT :]

            # ---- scalar (HW DGE) queue: the other weight stream --------
            # first pairs in small pieces: each completion semaphore has
            # multi-us latency, so finer pieces unblock L2(0)'s first
            # m-blocks sooner.
            for mb in range(4):
                nc.scalar.dma_start(
                    w2_sb[:, 0:2, mb * 256 : (mb + 1) * 256],
                    w2[:, 0:2, mb * 256 : (mb + 1) * 256],
                )
            nc.scalar.dma_start(w2_sb[:, 2:4, :512], w2[:, 2:4, :512])
            nc.scalar.dma_start(w2_sb[:, 2:4, 512:], w2[:, 2:4, 512:])
            w3_sb = wp.tile([P, KT, H], F8, tag="w3")
            nc.scalar.dma_start(w3_sb[:, :4, :], w3[:, :4, :])
            nc.scalar.dma_start(w3_sb[:, 4:, :], w3[:, 4:, :])

            # ---- gpsimd queue: only the tiny L4 weight ----------------
            w4_sb = wp.tile([P, KT, 16], F8, tag="w4")
            nc.gpsimd.dma_start(w4_sb[:], w4[:])

            # ---- ReLU store: even m on ACT, odd m on DVE ---------------
            def relu_store(m, ps, h_t, b_sb, boff):
                dst = h_t[:, m : m + 1, :]
                if b_sb is None:
                    if m % 2 == 1:
                        nc.vector.tensor_single_scalar(dst, ps[:], 0.0, ALU.max)
                    else:
                        nc.scalar.activation(dst, ps[:], AF.Relu)
                    return
                bcol = b_sb[:, boff + m : boff + m + 1]
                if m % 2 == 1:
                    nc.vector.tensor_scalar(dst, ps[:], bcol, 0.0, ALU.add, ALU.max)
                else:
                    nc.scalar.activation(dst, ps[:], AF.Relu, bias=bcol)

            h_tiles = {}

            def emit_l1_group(c, g):
                # 4 K=8 matmuls packed into the PE row groups (x/W1 are
                # host-replicated at partitions 0/32/64/96).
                key = ("h1", c)
                if key not in h_tiles:
                    h_tiles[key] = hp.tile(
                        [P, KT, NB], F8, tag="h1", name=f"h1_{c}", bufs=3
                    )
                h1_t = h_tiles[key]
                pss = []
                for i in range(4):
                    m = g * 4 + i
                    r0 = 32 * i
                    ps = pp.tile([P, NB], F32, tag="ps", name=f"ps1_{c}_{m}")
                    nc.tensor.matmul(
                        ps[:],
                        w1_sb[r0 : r0 + D1, m * P : (m + 1) * P],
                        wx_sb[r0 : r0 + D1, H + c * NB : H + (c + 1) * NB],
                        start=True,
                        stop=True,
                        tile_position=(r0, 0),
                    )
                    pss.append(ps)
                for i in range(4):
                    relu_store(g * 4 + i, pss[i], h1_t, None, 0)

            def emit_l1(c):
                emit_l1_group(c, 0)
                emit_l1_group(c, 1)

            def emit_hidden(c, lname, h_in, h_out, w_sb, b_sb, boff, mid=None):
                for m in range(KT):
                    ps = pp.tile([P, NB], F32, tag="ps", name=f"ps{lname}_{c}_{m}")
                    for q in range(QT):
                        nc.tensor.matmul(
                            ps[:],
                            w_sb[:, 2 * q : 2 * q + 2, m * P : (m + 1) * P],
                            h_in[:, 2 * q : 2 * q + 2, :],
                            start=(q == 0),
                            stop=(q == QT - 1),
                            perf_mode=DR,
                        )
                    relu_store(m, ps, h_out, b_sb, boff)
                    if m == 3 and mid is not None:
                        mid()

            # ---- L1 software-pipelined two chunks ahead ----------------
            emit_l1(0)
            emit_l1(1)

            # ---- constraint bounds from x (independent of the MLP).
            # Emitted here so the DVE-queue work lands after chunk 0/1's h1
            # ReLUs but well before the first raw conversion needs `ub`.
            def col(c):
                return xc_sb[:, c * JT : (c + 1) * JT]

            cem, slag, fly, wat, ager = col(0), col(1), col(2), col(3), col(7)

            def ctile(name):
                return cp.tile([P, JT], F32, tag=name, name=name)

            def mtile(name):
                return cp.tile([P, JT], mybir.dt.uint8, tag=name, name=name)

            vec = nc.vector

            age = ctile("age")
            vec.tensor_single_scalar(age[:], ager, 1.0, ALU.max)
            cmask = mtile("cmask")
            vec.tensor_single_scalar(cmask[:], cem, 0.0, ALU.is_gt)
            wmask = mtile("wmask")
            vec.tensor_single_scalar(wmask[:], wat, 0.0, ALU.is_gt)
            vmask = mtile("vmask")
            vec.tensor_tensor(vmask[:], cmask[:], wmask[:], ALU.bitwise_and)
            ones = ctile("ones")
            vec.memset(ones[:], 1.0)
            cems = ctile("cems")
            vec.select(cems[:], cmask[:], cem, ones[:])
            rcem = ctile("rcem")
            vec.reciprocal(rcem[:], cems[:])
            wc = ctile("wc")
            vec.tensor_tensor(wc[:], wat, rcem[:], ALU.mult)
            scm = ctile("scm")
            vec.tensor_tensor(scm[:], slag, fly, ALU.add)
            binder = ctile("binder")
            vec.tensor_tensor(binder[:], cem, scm[:], ALU.add)
            den1 = ctile("den1")
            vec.tensor_single_scalar(den1[:], binder[:], 0.1, ALU.max)
            rden1 = ctile("rden1")
            vec.reciprocal(rden1[:], den1[:])
            r1s = ctile("r1s")
            vec.tensor_tensor(r1s[:], scm[:], rden1[:], ALU.mult)
            amax = ctile("amax")
            vec.tensor_scalar(amax[:], r1s[:], -0.15, 0.95, ALU.mult, ALU.add)
            hyd = ctile("hyd")
            vec.tensor_single_scalar(hyd[:], wc[:], 1.0, ALU.add)
            rhyd = ctile("rhyd")
            vec.reciprocal(rhyd[:], hyd[:])
            ea = ctile("ea")
            vec.tensor_tensor(ea[:], rhyd[:], age[:], ALU.mult)
            ex = ctile("ex")
            nc.scalar.activation(ex[:], ea[:], AF.Exp, scale=-0.01)
            omex = ctile("omex")
            vec.tensor_scalar(omex[:], ex[:], -1.0, 1.0, ALU.mult, ALU.add)
            alpha = ctile("alpha")
            vec.tensor_tensor(alpha[:], amax[:], omex[:], ALU.mult)
            bmask = mtile("bmask")
            vec.tensor_single_scalar(bmask[:], binder[:], 0.0, ALU.is_gt)
            bsafe = ctile("bsafe")
            vec.select(bsafe[:], bmask[:], binder[:], ones[:])
            rbs = ctile("rbs")
            vec.reciprocal(rbs[:], bsafe[:])
            cf = ctile("cf")
            vec.tensor_tensor(cf[:], cem, rbs[:], ALU.mult)
            acf = ctile("acf")
            vec.tensor_tensor(acf[:], alpha[:], cf[:], ALU.mult)
            wcmask = mtile("wcmask")
            vec.tensor_single_scalar(wcmask[:], wc[:], 0.0, ALU.is_gt)
            wcsafe = ctile("wcsafe")
            vec.select(wcsafe[:], wcmask[:], wc[:], ones[:])
            rwcs = ctile("rwcs")
            vec.reciprocal(rwcs[:], wcsafe[:])
            gel = ctile("gel")
            vec.tensor_tensor(gel[:], acf[:], rwcs[:], ALU.mult)
            g = ctile("g")
            vec.tensor_scalar(g[:], gel[:], 0.01, 10.0, ALU.max, ALU.min)
            g2 = ctile("g2")
            vec.tensor_tensor(g2[:], g[:], g[:], ALU.mult)
            g3 = ctile("g3")
            vec.tensor_tensor(g3[:], g2[:], g[:], ALU.mult)
            phys = ctile("phys")
            vec.tensor_scalar(phys[:], g3[:], 50.0, 5.0, ALU.mult, ALU.max)
            physl = ctile("physl")
            vec.tensor_single_scalar(physl[:], phys[:], 120.0, ALU.min)
            tot1 = ctile("tot1")
            vec.tensor_tensor(tot1[:], cem, wat, ALU.add)
            total = ctile("total")
            vec.tensor_tensor(total[:], tot1[:], scm[:], ALU.add)
            dtot = ctile("dtot")
            vec.tensor_single_scalar(dtot[:], total[:], 1e-6, ALU.max)
            rtot = ctile("rtot")
            vec.reciprocal(rtot[:], dtot[:])
            cfac = ctile("cfac")
            vec.tensor_tensor(cfac[:], cem, rtot[:], ALU.mult)
            cons = ctile("cons")
            vec.tensor_single_scalar(cons[:], cfac[:], 120.0, ALU.mult)
            ub = ctile("ub")
            vec.tensor_tensor(ub[:], physl[:], cons[:], ALU.min)
            amask = mtile("amask")
            vec.tensor_tensor(amask[:], vmask[:], bmask[:], ALU.bitwise_and)

            # ---- raw [1, w] -> batch-major clamp -> store --------------
            raw_bm = cp.tile([P, JT], F32, tag="raw_bm")
            rawb = ctile("rawb")
            lo5 = ctile("lo5")
            constr = ctile("constr")
            outsb = cp.tile([P, JT], F32, tag="outsb")
            nj = NB // P  # batch-major columns produced per chunk

            def raw_to_out(c, ps_part, rows, part_id, eng=None):
                # batch-major mapping: sample n of chunk c lives at
                # (partition n//4, column c*4 + n%4).  `rows` selects the
                # partition range this piece covers (full or half chunk).
                eng = eng or nc.sync
                w = (rows.stop - rows.start) * nj
                rawt = rp.tile([1, w], F32, tag="rawt", name=f"rawt{c}_{part_id}")
                vec.tensor_copy(rawt[:], ps_part)
                sl = slice(c * nj, (c + 1) * nj)
                # single strided SBUF->SBUF DMA does the [1, w] -> [p, 4]
                # batch-major transpose in one hop.
                eng.dma_start(
                    raw_bm[rows, sl],
                    rawt[0:1, :].rearrange("o (p j) -> o p j", j=nj),
                )
                vec.tensor_scalar(
                    rawb[rows, sl], raw_bm[rows, sl], 1.0 / RAW_DIV,
                    b4_sb[rows, 0:1], ALU.mult, ALU.add,
                )
                vec.tensor_single_scalar(lo5[rows, sl], rawb[rows, sl], 5.0, ALU.max)
                vec.tensor_tensor(
                    constr[rows, sl], lo5[rows, sl], ub[rows, sl], ALU.min
                )
                vec.select(
                    outsb[rows, sl], amask[rows, sl], constr[rows, sl],
                    rawb[rows, sl],
                )
                nc.gpsimd.dma_start(out_d[rows, sl], outsb[rows, sl])

            def raw_tr_out(c, ps_part, j0, part_id):
                # last-chunk path: PE-transpose [1, 256] -> two PSUM columns
                # (no DMA, no completion-semaphore latency in the tail).
                # mapping for this chunk: sample n -> (p = n % 128,
                # column c*4 + n // 128).
                w = 2 * P
                rawt = rp.tile([1, w], F32, tag="rawt", name=f"rawtT{c}_{part_id}")
                nc.scalar.copy(rawt[:], ps_part)
                psT = pp.tile([P, 4], F32, tag="ps", name=f"psT{c}_{part_id}")
                for jj in range(2):
                    nc.tensor.matmul(
                        psT[:, j0 + jj : j0 + jj + 1],
                        rawt[0:1, jj * P : (jj + 1) * P],
                        ones[0:1, 0:1],
                        is_transpose=True,
                    )
                sl = slice(c * nj + j0, c * nj + j0 + 2)
                vec.tensor_scalar(
                    rawb[:, sl], psT[:, j0 : j0 + 2], 1.0 / RAW_DIV,
                    b4_sb[:, 0:1], ALU.mult, ALU.add,
                )
                vec.tensor_single_scalar(lo5[:, sl], rawb[:, sl], 5.0, ALU.max)
                vec.tensor_tensor(constr[:, sl], lo5[:, sl], ub[:, sl], ALU.min)
                vec.select(outsb[:, sl], amask[:, sl], constr[:, sl], rawb[:, sl])
                nc.sync.dma_start(out_d[:, sl], outsb[:, sl])

            # ---- main chunk loop ---------------------------------------
            for c in range(NCH):
                h1_t = h_tiles[("h1", c)]
                h2_t = hp.tile([P, KT, NB], F8, tag="h2", name=f"h2_{c}", bufs=3)
                # L1 lookahead groups ride inside/after L2's m-loop so their
                # PSUM slots reuse buffers whose ReLUs finished long ago.
                mid = (lambda: emit_l1_group(c + 2, 0)) if c + 2 < NCH else None
                emit_hidden(c, "2", h1_t, h2_t, w2_sb, b23_sb, 0, mid=mid)
                if c + 2 < NCH:
                    emit_l1_group(c + 2, 1)
                h3_t = hp.tile([P, KT, NB], F8, tag="h3", name=f"h3_{c}", bufs=3)
                emit_hidden(c, "3", h2_t, h3_t, w3_sb, b23_sb, KT)

                if c < NCH - 1:
                    ps4 = pp.tile([2, NB], F32, tag="ps", name=f"ps4_{c}")
                    for q in range(QT):
                        nc.tensor.matmul(
                            ps4[:],
                            w4_sb[:, 2 * q : 2 * q + 2, 0:2],
                            h3_t[:, 2 * q : 2 * q + 2, :],
                            start=(q == 0),
                            stop=(q == QT - 1),
                            perf_mode=DR,
                        )
                    raw_to_out(c, ps4[0:1, :], slice(0, P), "a")
                else:
                    # last chunk: split L4 into halves so the first half's
                    # raw conversion overlaps the second half's matmuls.
                    HB = NB // 2
                    ps4a = pp.tile([2, HB], F32, tag="ps", name="ps4_la")
                    ps4b = pp.tile([2, HB], F32, tag="ps", name="ps4_lb")
                    for q in range(QT):
                        nc.tensor.matmul(
                            ps4a[:],
                            w4_sb[:, 2 * q : 2 * q + 2, 0:2],
                            h3_t[:, 2 * q : 2 * q + 2, :HB],
                            start=(q == 0),
                            stop=(q == QT - 1),
                            perf_mode=DR,
                        )
                    raw_tr_out(c, ps4a[0:1, :], 0, "a")
                    for q in range(QT):
                        nc.tensor.matmul(
                            ps4b[:],
                            w4_sb[:, 2 * q : 2 * q + 2, 0:2],
                            h3_t[:, 2 * q : 2 * q + 2, HB:],
                            start=(q == 0),
                            stop=(q == QT - 1),
                            perf_mode=DR,
                        )
                    raw_tr_out(c, ps4b[0:1, :], 2, "b")

    nc.compile()
    return nc


def _get_nc():
    if "nc" not in _CACHE:
        _CACHE["nc"] = _build_nc()
    return _CACHE["nc"]


def _prep_in_maps(x, W1, b1, W2, b2, W3, b3, W4, b4):
    f = np.float32
    f8 = mybir.dt.np(F8)
    x = np.ascontiguousarray(np.asarray(x, f))
    W1 = np.asarray(W1, f)
    W2 = np.asarray(W2, f)
    W3 = np.asarray(W3, f)
    W4 = np.asarray(W4, f)
    # [p, k, m] = W[k*128+p, m], scaled + quantized to fp8
    w2p = np.ascontiguousarray(
        (S2 * W2).reshape(KT, P, H).transpose(1, 0, 2).astype(f8)
    )
    w3p = np.ascontiguousarray(
        (S3 * W3).reshape(KT, P, H).transpose(1, 0, 2).astype(f8)
    )
    w4p = np.zeros((P, KT, 16), f8)
    w4c = (S4 * W4).reshape(KT, P).T.astype(f8)
    w4p[:, :, 0] = w4c
    w4p[:, :, 1] = w4c
    b1p = (S1 * np.asarray(b1, f)).reshape(KT, P).T
    b2p = (S1 * S2 * np.asarray(b2, f)).reshape(KT, P).T
    b3p = (S1 * S2 * S3 * np.asarray(b3, f)).reshape(KT, P).T
    b4p = np.full((P, 1), np.asarray(b4, f).reshape(-1)[0], f)
    bconsts_c = np.ascontiguousarray(np.concatenate([b1p, b2p, b3p, b4p], axis=1))

    in_maps = []
    for c in range(N_CORES):
        sl = x[c * BC : (c + 1) * BC]  # [4096, 8]
        # wx = [w1 | x^T] in bf16, host-replicated 4x for the PE row
        # groups.  bf16 x only feeds the MLP (clamp-protected); the
        # constraint path uses exact fp32 x via xc.
        bf = mybir.dt.np(BF16)
        wx1 = np.empty((D_IN + 1, H + BC), f)
        wx1[:D_IN, :H] = S1 * W1
        wx1[:D_IN, H:] = sl.T
        wx1[D_IN, :H] = S1 * np.asarray(b1, f)
        wx1[D_IN, H:] = 1.0
        wx_c = np.tile(wx1.astype(bf), (4, 1))
        # chunks 0-6: xc[p, f*JT + c*4 + j] = sl[c*512 + p*4 + j, f]
        # chunk 7 (PE-transpose path): xc[p, f*JT + 28 + j] = sl[3584 + j*128 + p, f]
        xc_c = (
            sl.reshape(NCH, P, NB // P, D_IN)
            .transpose(1, 3, 0, 2)
            .reshape(P, D_IN * JT)
            .copy()
        )
        last = sl[7 * NB :].reshape(NB // P, P, D_IN)  # [j, p, f]
        xcv = xc_c.reshape(P, D_IN, JT)
        xcv[:, :, 7 * (NB // P) :] = last.transpose(1, 2, 0)
        xc_c = np.ascontiguousarray(xc_c)
        in_maps.append(
            {
                "wx": np.ascontiguousarray(wx_c),
                "bconsts": bconsts_c,
                "xc": xc_c,
                "w2": w2p,
                "w3": w3p,
                "w4": w4p,
            }
        )
    return in_maps


def kernel(x, W1, b1, W2, b2, W3, b3, W4, b4, **run_kwargs):
    nc = _get_nc()
    in_maps = _prep_in_maps(x, W1, b1, W2, b2, W3, b3, W4, b4)
    res = run_bass_kernel_spmd(nc, in_maps, core_ids=list(range(N_CORES)), **run_kwargs)
    out = np.empty((B, 1), np.float32)
    for c in range(N_CORES):
        obm = res.results[c]["out_bm"].reshape(P, NCH, NB // P)
        core = obm.transpose(1, 0, 2).reshape(BC).copy()
        core[7 * NB :] = obm[:, 7, :].T.reshape(NB)
        out[c * BC : (c + 1) * BC, 0] = core
    if run_kwargs:
        kernel.last_results = res
    return out


# revision 25
# speedup vs baseline: 1.0267x; 1.0267x over previous
"""Trainium2 Bass kernel for the CementPINN MLP (dense_mlp, 8 cores).

Data-parallel: x [32768, 8] is sharded along batch into 8 shards of 4096
rows; MLP weights are replicated on every core.  Per core the MLP runs
feature-major (activations h^T [feat, batch]); every layer is
out^T[m] = sum_k W[k,m]^T @ h^T[k] with the weight tile stationary.

L2/L3/L4 matmuls run in fp8 (e4m3) with MatmulPerfMode.DoubleRow: each
instruction contracts a PAIR of 128-feature k-tiles (stationary [128,2,128],
moving [128,2,512]) at ~1.5x the fp32r MAC rate.  Host-side the weights are
pre-scaled by powers of two (W2x4, W3x4, W4x16) so the fp8 encoding stays in
the normal range; activations carry the compounded scale (h1'=4h1, h2'=16h2,
h3'=64h3) and each ReLU stage is one fused instruction:
  ACT:  relu(psum + b')        DVE: (psum + b') max 0   (biases pre-scaled)
The raw MLP output is psum/1024 + b4.  ReLU stages split evenly over the
scalar/vector engines.  L1 (K=9: x plus a ones-row that carries the bias,
bf16) is packed 4-wide into PE row groups via tile_position and
software-pipelined two chunks ahead (groups interleaved into L2's m-loop so
their PSUM slots reuse long-freed buffers).  Weights stream over both HW
DMA queues (SP + Activation) with the first pairs split small because each
completion semaphore carries multi-us latency.  The physics-constraint
clamp is computed batch-major on [128, 32] tiles from a host-pretransposed
copy of x; the per-chunk raw row [1, 512] is converted to batch-major with
a single SBUF->SBUF strided DMA for chunks 0-6 and via PE-transpose into
PSUM for the last chunk (keeps DMA-semaphore latency out of the tail).
"""

import numpy as np

import concourse.bacc as bacc
import concourse.mybir as mybir
import concourse.tile as tile
from concourse.bass_utils import run_bass_kernel_spmd

F32 = mybir.dt.float32
F32R = mybir.dt.float32r
F8 = mybir.dt.float8e4
BF16 = mybir.dt.bfloat16
AF = mybir.ActivationFunctionType
ALU = mybir.AluOpType
DR = mybir.MatmulPerfMode.DoubleRow

N_CORES = 8
B = 32768
BC = B // N_CORES  # 4096 rows per core
D_IN = 8
H = 1024
P = 128
NB = 512  # batch columns per chunk (= one fp32 PSUM bank)
NCH = BC // NB  # 8 chunks per core
KT = H // P  # 8 feature tiles
QT = KT // 2  # 4 k-tile pairs (DoubleRow)
JT = BC // P  # 32 batch-major columns

# weight pre-scales (powers of two; folded back out via biases / raw stage)
S1 = 4.0  # W1 *= 4 (fp32r, exact)     -> h1' = 4 h1
S2 = 4.0  # W2 *= 4 (fp8)              -> h2' = 16 h2
S3 = 4.0  # W3 *= 4 (fp8)              -> h3' = 64 h3
S4 = 16.0  # W4 *= 16 (fp8)            -> psum4 = 1024 * (W4^T h3)
RAW_DIV = S1 * S2 * S3 * S4  # 1024

_CACHE = {}


def _build_nc():
    nc = bacc.Bacc("TRN2", target_bir_lowering=False, debug=False)

    NBC = KT + 2 * KT + 1  # b1 | b23 | b4
    D1 = D_IN + 1  # extra contraction row carries the L1 bias
    wx = nc.declare_dram_parameter("wx", [4 * D1, H + BC], BF16, isOutput=False)
    bconsts = nc.declare_dram_parameter("bconsts", [P, NBC], F32, isOutput=False)
    xc = nc.declare_dram_parameter("xc", [P, D_IN * JT], F32, isOutput=False)
    w2 = nc.declare_dram_parameter("w2", [P, KT, H], F8, isOutput=False)
    w3 = nc.declare_dram_parameter("w3", [P, KT, H], F8, isOutput=False)
    w4 = nc.declare_dram_parameter("w4", [P, KT, 16], F8, isOutput=False)
    out_d = nc.declare_dram_parameter("out_bm", [P, JT], F32, isOutput=True)

    with tile.TileContext(nc) as tc:
        with (
            tc.tile_pool(name="wts", bufs=1) as wp,
            tc.tile_pool(name="xin", bufs=1) as xp,
            tc.tile_pool(name="acts", bufs=3) as hp,
            tc.tile_pool(name="raw", bufs=2) as rp,
            tc.tile_pool(name="cst", bufs=1) as cp,
            tc.tile_pool(name="ps", bufs=8, space="PSUM") as pp,
        ):
            # ---- sync (HW DGE) queue: biases first (ReLUs need them
            # early), then x+W1 in bf16, host-replicated into the 4 PE row
            # groups (one small DMA per group), then one W2 pair, then xc.
            # The scalar HW queue streams the other weights in parallel.
            wx_sb = wp.tile([P, H + BC], BF16, tag="wx")
            for g in range(4):
                r0 = 32 * g
                nc.sync.dma_start(
                    wx_sb[r0 : r0 + D1, :], wx[g * D1 : (g + 1) * D1, :]
                )
            bc_sb = cp.tile([P, NBC], F32, tag="bconsts")
            nc.sync.dma_start(bc_sb[:], bconsts[:])
            w2_sb = wp.tile([P, KT, H], F8, tag="w2")
            nc.sync.dma_start(w2_sb[:, 4:6, :512], w2[:, 4:6, :512])
            nc.sync.dma_start(w2_sb[:, 4:6, 512:], w2[:, 4:6, 512:])
            nc.sync.dma_start(w2_sb[:, 6:8, :512], w2[:, 6:8, :512])
            nc.sync.dma_start(w2_sb[:, 6:8, 512:], w2[:, 6:8, 512:])
            xc_sb = cp.tile([P, D_IN * JT], F32, tag="xc")
            nc.sync.dma_start(xc_sb[:], xc[:])
            w1_sb = wx_sb[:, :H]
            b1_sb = bc_sb[:, :KT]
            b23_sb = bc_sb[:, KT : 3 * KT]
            b4_sb = bc_sb[:, 3 * KT :]

            # ---- scalar (HW DGE) queue: the other weight stream --------
            # first pairs in small pieces: each completion semaphore has
            # multi-us latency, so finer pieces unblock L2(0)'s first
            # m-blocks sooner.
            for mb in range(4):
                nc.scalar.dma_start(
                    w2_sb[:, 0:2, mb * 256 : (mb + 1) * 256],
                    w2[:, 0:2, mb * 256 : (mb + 1) * 256],
                )
            nc.scalar.dma_start(w2_sb[:, 2:4, :512], w2[:, 2:4, :512])
            nc.scalar.dma_start(w2_sb[:, 2:4, 512:], w2[:, 2:4, 512:])
            w3_sb = wp.tile([P, KT, H], F8, tag="w3")
            nc.scalar.dma_start(w3_sb[:, :4, :], w3[:, :4, :])
            nc.scalar.dma_start(w3_sb[:, 4:, :], w3[:, 4:, :])

            # ---- gpsimd queue: only the tiny L4 weight ----------------
            w4_sb = wp.tile([P, KT, 16], F8, tag="w4")
            nc.gpsimd.dma_start(w4_sb[:], w4[:])

            # ---- ReLU store: even m on ACT, odd m on DVE ---------------
            def relu_store(m, ps, h_t, b_sb, boff):
                dst = h_t[:, m : m + 1, :]
                if b_sb is None:
                    if m % 2 == 1:
                        nc.vector.tensor_single_scalar(dst, ps[:], 0.0, ALU.max)
                    else:
                        nc.scalar.activation(dst, ps[:], AF.Relu)
                    return
                bcol = b_sb[:, boff + m : boff + m + 1]
                if m % 2 == 1:
                    nc.vector.tensor_scalar(dst, ps[:], bcol, 0.0, ALU.add, ALU.max)
                else:
                    nc.scalar.activation(dst, ps[:], AF.Relu, bias=bcol)

            h_tiles = {}

            def emit_l1_group(c, g):
                # 4 K=8 matmuls packed into the PE row groups (x/W1 are
                # host-replicated at partitions 0/32/64/96).
                key = ("h1", c)
                if key not in h_tiles:
                    h_tiles[key] = hp.tile(
                        [P, KT, NB], F8, tag="h1", name=f"h1_{c}", bufs=3
                    )
                h1_t = h_tiles[key]
                pss = []
                for i in range(4):
                    m = g * 4 + i
                    # chunk 0 runs unpacked on PE rows 0-7: it then depends
                    # only on the FIRST wx row-group DMA semaphore, so the
                    # PE starts ~1.3us earlier; the extra runtime fills the
                    # weight-wait window.
                    r0 = 0 if c == 0 else 32 * i
                    ps = pp.tile([P, NB], F32, tag="ps", name=f"ps1_{c}_{m}")
                    nc.tensor.matmul(
                        ps[:],
                        w1_sb[r0 : r0 + D1, m * P : (m + 1) * P],
                        wx_sb[r0 : r0 + D1, H + c * NB : H + (c + 1) * NB],
                        start=True,
                        stop=True,
                        tile_position=(r0, 0) if c > 0 else None,
                    )
                    pss.append(ps)
                for i in range(4):
                    relu_store(g * 4 + i, pss[i], h1_t, None, 0)

            def emit_l1(c):
                emit_l1_group(c, 0)
                emit_l1_group(c, 1)

            def emit_hidden(c, lname, h_in, h_out, w_sb, b_sb, boff, mid=None):
                for m in range(KT):
                    ps = pp.tile([P, NB], F32, tag="ps", name=f"ps{lname}_{c}_{m}")
                    for q in range(QT):
                        nc.tensor.matmul(
                            ps[:],
                            w_sb[:, 2 * q : 2 * q + 2, m * P : (m + 1) * P],
                            h_in[:, 2 * q : 2 * q + 2, :],
                            start=(q == 0),
                            stop=(q == QT - 1),
                            perf_mode=DR,
                        )
                    relu_store(m, ps, h_out, b_sb, boff)
                    if m == 3 and mid is not None:
                        mid()

            # ---- L1 software-pipelined two chunks ahead ----------------
            emit_l1(0)
            emit_l1(1)

            # ---- constraint bounds from x (independent of the MLP).
            # Emitted here so the DVE-queue work lands after chunk 0/1's h1
            # ReLUs but well before the first raw conversion needs `ub`.
            def col(c):
                return xc_sb[:, c * JT : (c + 1) * JT]

            cem, slag, fly, wat, ager = col(0), col(1), col(2), col(3), col(7)

            def ctile(name):
                return cp.tile([P, JT], F32, tag=name, name=name)

            def mtile(name):
                return cp.tile([P, JT], mybir.dt.uint8, tag=name, name=name)

            vec = nc.vector

            age = ctile("age")
            vec.tensor_single_scalar(age[:], ager, 1.0, ALU.max)
            cmask = mtile("cmask")
            vec.tensor_single_scalar(cmask[:], cem, 0.0, ALU.is_gt)
            wmask = mtile("wmask")
            vec.tensor_single_scalar(wmask[:], wat, 0.0, ALU.is_gt)
            vmask = mtile("vmask")
            vec.tensor_tensor(vmask[:], cmask[:], wmask[:], ALU.bitwise_and)
            ones = ctile("ones")
            vec.memset(ones[:], 1.0)
            cems = ctile("cems")
            vec.select(cems[:], cmask[:], cem, ones[:])
            rcem = ctile("rcem")
            vec.reciprocal(rcem[:], cems[:])
            wc = ctile("wc")
            vec.tensor_tensor(wc[:], wat, rcem[:], ALU.mult)
            scm = ctile("scm")
            vec.tensor_tensor(scm[:], slag, fly, ALU.add)
            binder = ctile("binder")
            vec.tensor_tensor(binder[:], cem, scm[:], ALU.add)
            den1 = ctile("den1")
            vec.tensor_single_scalar(den1[:], binder[:], 0.1, ALU.max)
            rden1 = ctile("rden1")
            vec.reciprocal(rden1[:], den1[:])
            r1s = ctile("r1s")
            vec.tensor_tensor(r1s[:], scm[:], rden1[:], ALU.mult)
            amax = ctile("amax")
            vec.tensor_scalar(amax[:], r1s[:], -0.15, 0.95, ALU.mult, ALU.add)
            hyd = ctile("hyd")
            vec.tensor_single_scalar(hyd[:], wc[:], 1.0, ALU.add)
            rhyd = ctile("rhyd")
            vec.reciprocal(rhyd[:], hyd[:])
            ea = ctile("ea")
            vec.tensor_tensor(ea[:], rhyd[:], age[:], ALU.mult)
            ex = ctile("ex")
            nc.scalar.activation(ex[:], ea[:], AF.Exp, scale=-0.01)
            omex = ctile("omex")
            vec.tensor_scalar(omex[:], ex[:], -1.0, 1.0, ALU.mult, ALU.add)
            alpha = ctile("alpha")
            vec.tensor_tensor(alpha[:], amax[:], omex[:], ALU.mult)
            bmask = mtile("bmask")
            vec.tensor_single_scalar(bmask[:], binder[:], 0.0, ALU.is_gt)
            bsafe = ctile("bsafe")
            vec.select(bsafe[:], bmask[:], binder[:], ones[:])
            rbs = ctile("rbs")
            vec.reciprocal(rbs[:], bsafe[:])
            cf = ctile("cf")
            vec.tensor_tensor(cf[:], cem, rbs[:], ALU.mult)
            acf = ctile("acf")
            vec.tensor_tensor(acf[:], alpha[:], cf[:], ALU.mult)
            wcmask = mtile("wcmask")
            vec.tensor_single_scalar(wcmask[:], wc[:], 0.0, ALU.is_gt)
            wcsafe = ctile("wcsafe")
            vec.select(wcsafe[:], wcmask[:], wc[:], ones[:])
            rwcs = ctile("rwcs")
            vec.reciprocal(rwcs[:], wcsafe[:])
            gel = ctile("gel")
            vec.tensor_tensor(gel[:], acf[:], rwcs[:], ALU.mult)
            g = ctile("g")
            vec.tensor_scalar(g[:], gel[:], 0.01, 10.0, ALU.max, ALU.min)
            g2 = ctile("g2")
            vec.tensor_tensor(g2[:], g[:], g[:], ALU.mult)
            g3 = ctile("g3")
            vec.tensor_tensor(g3[:], g2[:], g[:], ALU.mult)
            phys = ctile("phys")
            vec.tensor_scalar(phys[:], g3[:], 50.0, 5.0, ALU.mult, ALU.max)
            physl = ctile("physl")
            vec.tensor_single_scalar(physl[:], phys[:], 120.0, ALU.min)
            tot1 = ctile("tot1")
            vec.tensor_tensor(tot1[:], cem, wat, ALU.add)
            total = ctile("total")
            vec.tensor_tensor(total[:], tot1[:], scm[:], ALU.add)
            dtot = ctile("dtot")
            vec.tensor_single_scalar(dtot[:], total[:], 1e-6, ALU.max)
            rtot = ctile("rtot")
            vec.reciprocal(rtot[:], dtot[:])
            cfac = ctile("cfac")
            vec.tensor_tensor(cfac[:], cem, rtot[:], ALU.mult)
            cons = ctile("cons")
            vec.tensor_single_scalar(cons[:], cfac[:], 120.0, ALU.mult)
            ub = ctile("ub")
            vec.tensor_tensor(ub[:], physl[:], cons[:], ALU.min)
            amask = mtile("amask")
            vec.tensor_tensor(amask[:], vmask[:], bmask[:], ALU.bitwise_and)

            # ---- raw [1, w] -> batch-major clamp -> store --------------
            raw_bm = cp.tile([P, JT], F32, tag="raw_bm")
            rawb = ctile("rawb")
            lo5 = ctile("lo5")
            constr = ctile("constr")
            outsb = cp.tile([P, JT], F32, tag="outsb")
            nj = NB // P  # batch-major columns produced per chunk

            def raw_to_out(c, ps_part, rows, part_id, eng=None):
                # batch-major mapping: sample n of chunk c lives at
                # (partition n//4, column c*4 + n%4).  `rows` selects the
                # partition range this piece covers (full or half chunk).
                eng = eng or nc.sync
                w = (rows.stop - rows.start) * nj
                rawt = rp.tile([1, w], F32, tag="rawt", name=f"rawt{c}_{part_id}")
                vec.tensor_copy(rawt[:], ps_part)
                sl = slice(c * nj, (c + 1) * nj)
                # single strided SBUF->SBUF DMA does the [1, w] -> [p, 4]
                # batch-major transpose in one hop.
                eng.dma_start(
                    raw_bm[rows, sl],
                    rawt[0:1, :].rearrange("o (p j) -> o p j", j=nj),
                )
                vec.tensor_scalar(
                    rawb[rows, sl], raw_bm[rows, sl], 1.0 / RAW_DIV,
                    b4_sb[rows, 0:1], ALU.mult, ALU.add,
                )
                vec.tensor_single_scalar(lo5[rows, sl], rawb[rows, sl], 5.0, ALU.max)
                vec.tensor_tensor(
                    constr[rows, sl], lo5[rows, sl], ub[rows, sl], ALU.min
                )
                vec.select(
                    outsb[rows, sl], amask[rows, sl], constr[rows, sl],
                    rawb[rows, sl],
                )
                nc.gpsimd.dma_start(out_d[rows, sl], outsb[rows, sl])

            def raw_tr_out(c, ps_part, j0, part_id):
                # last-chunk path: PE-transpose [1, 256] -> two PSUM columns
                # (no DMA, no completion-semaphore latency in the tail).
                # mapping for this chunk: sample n -> (p = n % 128,
                # column c*4 + n // 128).
                w = 2 * P
                rawt = rp.tile([1, w], F32, tag="rawt", name=f"rawtT{c}_{part_id}")
                nc.scalar.copy(rawt[:], ps_part)
                psT = pp.tile([P, 4], F32, tag="ps", name=f"psT{c}_{part_id}")
                for jj in range(2):
                    nc.tensor.matmul(
                        psT[:, j0 + jj : j0 + jj + 1],
                        rawt[0:1, jj * P : (jj + 1) * P],
                        ones[0:1, 0:1],
                        is_transpose=True,
                    )
                sl = slice(c * nj + j0, c * nj + j0 + 2)
                vec.tensor_scalar(
                    rawb[:, sl], psT[:, j0 : j0 + 2], 1.0 / RAW_DIV,
                    b4_sb[:, 0:1], ALU.mult, ALU.add,
                )
                vec.tensor_single_scalar(lo5[:, sl], rawb[:, sl], 5.0, ALU.max)
                vec.tensor_tensor(constr[:, sl], lo5[:, sl], ub[:, sl], ALU.min)
                vec.select(outsb[:, sl], amask[:, sl], constr[:, sl], rawb[:, sl])
                nc.sync.dma_start(out_d[:, sl], outsb[:, sl])

            # ---- main chunk loop ---------------------------------------
            for c in range(NCH):
                h1_t = h_tiles[("h1", c)]
                h2_t = hp.tile([P, KT, NB], F8, tag="h2", name=f"h2_{c}", bufs=3)
                # L1 lookahead groups ride inside/after L2's m-loop so their
                # PSUM slots reuse buffers whose ReLUs finished long ago.
                mid = (lambda: emit_l1_group(c + 2, 0)) if c + 2 < NCH else None
                emit_hidden(c, "2", h1_t, h2_t, w2_sb, b23_sb, 0, mid=mid)
                if c + 2 < NCH:
                    emit_l1_group(c + 2, 1)
                h3_t = hp.tile([P, KT, NB], F8, tag="h3", name=f"h3_{c}", bufs=3)
                emit_hidden(c, "3", h2_t, h3_t, w3_sb, b23_sb, KT)

                if c < NCH - 1:
                    ps4 = pp.tile([2, NB], F32, tag="ps", name=f"ps4_{c}")
                    for q in range(QT):
                        nc.tensor.matmul(
                            ps4[:],
                            w4_sb[:, 2 * q : 2 * q + 2, 0:2],
                            h3_t[:, 2 * q : 2 * q + 2, :],
                            start=(q == 0),
                            stop=(q == QT - 1),
                            perf_mode=DR,
                        )
                    raw_to_out(c, ps4[0:1, :], slice(0, P), "a")
                else:
                    # last chunk: split L4 into halves so the first half's
                    # raw conversion overlaps the second half's matmuls.
                    HB = NB // 2
                    ps4a = pp.tile([2, HB], F32, tag="ps", name="ps4_la")
                    ps4b = pp.tile([2, HB], F32, tag="ps", name="ps4_lb")
                    for q in range(QT):
                        nc.tensor.matmul(
                            ps4a[:],
                            w4_sb[:, 2 * q : 2 * q + 2, 0:2],
                            h3_t[:, 2 * q : 2 * q + 2, :HB],
                            start=(q == 0),
                            stop=(q == QT - 1),
                            perf_mode=DR,
                        )
                    raw_tr_out(c, ps4a[0:1, :], 0, "a")
                    for q in range(QT):
                        nc.tensor.matmul(
                            ps4b[:],
                            w4_sb[:, 2 * q : 2 * q + 2, 0:2],
                            h3_t[:, 2 * q : 2 * q + 2, HB:],
                            start=(q == 0),
                            stop=(q == QT - 1),
                            perf_mode=DR,
                        )
                    raw_tr_out(c, ps4b[0:1, :], 2, "b")

    nc.compile()
    return nc


def _get_nc():
    if "nc" not in _CACHE:
        _CACHE["nc"] = _build_nc()
    return _CACHE["nc"]


def _prep_in_maps(x, W1, b1, W2, b2, W3, b3, W4, b4):
    f = np.float32
    f8 = mybir.dt.np(F8)
    x = np.ascontiguousarray(np.asarray(x, f))
    W1 = np.asarray(W1, f)
    W2 = np.asarray(W2, f)
    W3 = np.asarray(W3, f)
    W4 = np.asarray(W4, f)
    # [p, k, m] = W[k*128+p, m], scaled + quantized to fp8
    w2p = np.ascontiguousarray(
        (S2 * W2).reshape(KT, P, H).transpose(1, 0, 2).astype(f8)
    )
    w3p = np.ascontiguousarray(
        (S3 * W3).reshape(KT, P, H).transpose(1, 0, 2).astype(f8)
    )
    w4p = np.zeros((P, KT, 16), f8)
    w4c = (S4 * W4).reshape(KT, P).T.astype(f8)
    w4p[:, :, 0] = w4c
    w4p[:, :, 1] = w4c
    b1p = (S1 * np.asarray(b1, f)).reshape(KT, P).T
    b2p = (S1 * S2 * np.asarray(b2, f)).reshape(KT, P).T
    b3p = (S1 * S2 * S3 * np.asarray(b3, f)).reshape(KT, P).T
    b4p = np.full((P, 1), np.asarray(b4, f).reshape(-1)[0], f)
    bconsts_c = np.ascontiguousarray(np.concatenate([b1p, b2p, b3p, b4p], axis=1))

    in_maps = []
    for c in range(N_CORES):
        sl = x[c * BC : (c + 1) * BC]  # [4096, 8]
        # wx = [w1 | x^T] in bf16, host-replicated 4x for the PE row
        # groups.  bf16 x only feeds the MLP (clamp-protected); the
        # constraint path uses exact fp32 x via xc.
        bf = mybir.dt.np(BF16)
        wx1 = np.empty((D_IN + 1, H + BC), f)
        wx1[:D_IN, :H] = S1 * W1
        wx1[:D_IN, H:] = sl.T
        wx1[D_IN, :H] = S1 * np.asarray(b1, f)
        wx1[D_IN, H:] = 1.0
        wx_c = np.tile(wx1.astype(bf), (4, 1))
        # chunks 0-6: xc[p, f*JT + c*4 + j] = sl[c*512 + p*4 + j, f]
        # chunk 7 (PE-transpose path): xc[p, f*JT + 28 + j] = sl[3584 + j*128 + p, f]
        xc_c = (
            sl.reshape(NCH, P, NB // P, D_IN)
            .transpose(1, 3, 0, 2)
            .reshape(P, D_IN * JT)
            .copy()
        )
        last = sl[7 * NB :].reshape(NB // P, P, D_IN)  # [j, p, f]
        xcv = xc_c.reshape(P, D_IN, JT)
        xcv[:, :, 7 * (NB // P) :] = last.transpose(1, 2, 0)
        xc_c = np.ascontiguousarray(xc_c)
        in_maps.append(
            {
                "wx": np.ascontiguousarray(wx_c),
                "bconsts": bconsts_c,
                "xc": xc_c,
                "w2": w2p,
                "w3": w3p,
                "w4": w4p,
            }
        )
    return in_maps


def kernel(x, W1, b1, W2, b2, W3, b3, W4, b4, **run_kwargs):
    nc = _get_nc()
    in_maps = _prep_in_maps(x, W1, b1, W2, b2, W3, b3, W4, b4)
    res = run_bass_kernel_spmd(nc, in_maps, core_ids=list(range(N_CORES)), **run_kwargs)
    out = np.empty((B, 1), np.float32)
    for c in range(N_CORES):
        obm = res.results[c]["out_bm"].reshape(P, NCH, NB // P)
        core = obm.transpose(1, 0, 2).reshape(BC).copy()
        core[7 * NB :] = obm[:, 7, :].T.reshape(NB)
        out[c * BC : (c + 1) * BC, 0] = core
    if run_kwargs:
        kernel.last_results = res
    return out


# revision 26
# speedup vs baseline: 1.0287x; 1.0020x over previous
"""Trainium2 Bass kernel for the CementPINN MLP (dense_mlp, 8 cores).

Data-parallel: x [32768, 8] is sharded along batch into 8 shards of 4096
rows; MLP weights are replicated on every core.  Per core the MLP runs
feature-major (activations h^T [feat, batch]); every layer is
out^T[m] = sum_k W[k,m]^T @ h^T[k] with the weight tile stationary.

L2/L3/L4 matmuls run in fp8 (e4m3) with MatmulPerfMode.DoubleRow: each
instruction contracts a PAIR of 128-feature k-tiles (stationary [128,2,128],
moving [128,2,512]) at ~1.5x the fp32r MAC rate.  Host-side the weights are
pre-scaled by powers of two (W2x4, W3x4, W4x16) so the fp8 encoding stays in
the normal range; activations carry the compounded scale (h1'=4h1, h2'=16h2,
h3'=64h3) and each ReLU stage is one fused instruction:
  ACT:  relu(psum + b')        DVE: (psum + b') max 0   (biases pre-scaled)
The raw MLP output is psum/1024 + b4.  ReLU stages split evenly over the
scalar/vector engines.  L1 (K=9: x plus a ones-row that carries the bias,
bf16) is packed 4-wide into PE row groups via tile_position and
software-pipelined two chunks ahead (groups interleaved into L2's m-loop so
their PSUM slots reuse long-freed buffers).  Weights stream over both HW
DMA queues (SP + Activation) with the first pairs split small because each
completion semaphore carries multi-us latency.  The physics-constraint
clamp is computed batch-major on [128, 32] tiles from a host-pretransposed
copy of x; the per-chunk raw row [1, 512] is converted to batch-major with
a single SBUF->SBUF strided DMA for chunks 0-6 and via PE-transpose into
PSUM for the last chunk (keeps DMA-semaphore latency out of the tail).
"""

import numpy as np

import concourse.bacc as bacc
import concourse.mybir as mybir
import concourse.tile as tile
from concourse.bass_utils import run_bass_kernel_spmd

F32 = mybir.dt.float32
F32R = mybir.dt.float32r
F8 = mybir.dt.float8e4
BF16 = mybir.dt.bfloat16
AF = mybir.ActivationFunctionType
ALU = mybir.AluOpType
DR = mybir.MatmulPerfMode.DoubleRow

N_CORES = 8
B = 32768
BC = B // N_CORES  # 4096 rows per core
D_IN = 8
H = 1024
P = 128
NB = 512  # batch columns per chunk (= one fp32 PSUM bank)
NCH = BC // NB  # 8 chunks per core
KT = H // P  # 8 feature tiles
QT = KT // 2  # 4 k-tile pairs (DoubleRow)
JT = BC // P  # 32 batch-major columns

# weight pre-scales (powers of two; folded back out via biases / raw stage)
S1 = 4.0  # W1 *= 4 (fp32r, exact)     -> h1' = 4 h1
S2 = 4.0  # W2 *= 4 (fp8)              -> h2' = 16 h2
S3 = 4.0  # W3 *= 4 (fp8)              -> h3' = 64 h3
S4 = 16.0  # W4 *= 16 (fp8)            -> psum4 = 1024 * (W4^T h3)
RAW_DIV = S1 * S2 * S3 * S4  # 1024

_CACHE = {}


def _build_nc():
    nc = bacc.Bacc("TRN2", target_bir_lowering=False, debug=False)

    NBC = KT + 2 * KT + 1  # b1 | b23 | b4
    D1 = D_IN + 1  # extra contraction row carries the L1 bias
    wx = nc.declare_dram_parameter("wx", [4 * D1, H + BC], BF16, isOutput=False)
    bconsts = nc.declare_dram_parameter("bconsts", [P, NBC], F32, isOutput=False)
    xc = nc.declare_dram_parameter("xc", [P, D_IN * JT], F32, isOutput=False)
    w2 = nc.declare_dram_parameter("w2", [P, KT, H], F8, isOutput=False)
    w3 = nc.declare_dram_parameter("w3", [P, KT, H], F8, isOutput=False)
    w4 = nc.declare_dram_parameter("w4", [P, KT, 16], F8, isOutput=False)
    out_d = nc.declare_dram_parameter("out_bm", [P, JT], F32, isOutput=True)

    with tile.TileContext(nc) as tc:
        with (
            tc.tile_pool(name="wts", bufs=1) as wp,
            tc.tile_pool(name="xin", bufs=1) as xp,
            tc.tile_pool(name="acts", bufs=3) as hp,
            tc.tile_pool(name="raw", bufs=2) as rp,
            tc.tile_pool(name="cst", bufs=1) as cp,
            tc.tile_pool(name="ps", bufs=8, space="PSUM") as pp,
        ):
            # ---- sync (HW DGE) queue: biases first (ReLUs need them
            # early), then x+W1 in bf16, host-replicated into the 4 PE row
            # groups (one small DMA per group), then one W2 pair, then xc.
            # The scalar HW queue streams the other weights in parallel.
            wx_sb = wp.tile([P, H + BC], BF16, tag="wx")
            for g in range(4):
                r0 = 32 * g
                nc.sync.dma_start(
                    wx_sb[r0 : r0 + D1, :], wx[g * D1 : (g + 1) * D1, :]
                )
            bc_sb = cp.tile([P, NBC], F32, tag="bconsts")
            nc.sync.dma_start(bc_sb[:], bconsts[:])
            w2_sb = wp.tile([P, KT, H], F8, tag="w2")
            nc.sync.dma_start(w2_sb[:, 4:6, :512], w2[:, 4:6, :512])
            nc.sync.dma_start(w2_sb[:, 4:6, 512:], w2[:, 4:6, 512:])
            nc.sync.dma_start(w2_sb[:, 6:8, :512], w2[:, 6:8, :512])
            nc.sync.dma_start(w2_sb[:, 6:8, 512:], w2[:, 6:8, 512:])
            xc_sb = cp.tile([P, D_IN * JT], F32, tag="xc")
            nc.sync.dma_start(xc_sb[:], xc[:])
            w1_sb = wx_sb[:, :H]
            b1_sb = bc_sb[:, :KT]
            b23_sb = bc_sb[:, KT : 3 * KT]
            b4_sb = bc_sb[:, 3 * KT :]

            # ---- scalar (HW DGE) queue: the other weight stream --------
            # first pairs in small pieces: each completion semaphore has
            # multi-us latency, so finer pieces unblock L2(0)'s first
            # m-blocks sooner.
            for mb in range(4):
                nc.scalar.dma_start(
                    w2_sb[:, 0:2, mb * 256 : (mb + 1) * 256],
                    w2[:, 0:2, mb * 256 : (mb + 1) * 256],
                )
            nc.scalar.dma_start(w2_sb[:, 2:4, :256], w2[:, 2:4, :256])
            nc.scalar.dma_start(w2_sb[:, 2:4, 256:512], w2[:, 2:4, 256:512])
            nc.scalar.dma_start(w2_sb[:, 2:4, 512:], w2[:, 2:4, 512:])
            w3_sb = wp.tile([P, KT, H], F8, tag="w3")
            nc.scalar.dma_start(w3_sb[:, :4, :], w3[:, :4, :])
            nc.scalar.dma_start(w3_sb[:, 4:, :], w3[:, 4:, :])

            # ---- gpsimd queue: only the tiny L4 weight ----------------
            w4_sb = wp.tile([P, KT, 16], F8, tag="w4")
            nc.gpsimd.dma_start(w4_sb[:], w4[:])

            # ---- ReLU store: even m on ACT, odd m on DVE ---------------
            def relu_store(m, ps, h_t, b_sb, boff):
                dst = h_t[:, m : m + 1, :]
                if b_sb is None:
                    if m % 2 == 1:
                        nc.vector.tensor_single_scalar(dst, ps[:], 0.0, ALU.max)
                    else:
                        nc.scalar.activation(dst, ps[:], AF.Relu)
                    return
                bcol = b_sb[:, boff + m : boff + m + 1]
                if m % 2 == 1:
                    nc.vector.tensor_scalar(dst, ps[:], bcol, 0.0, ALU.add, ALU.max)
                else:
                    nc.scalar.activation(dst, ps[:], AF.Relu, bias=bcol)

            h_tiles = {}

            def emit_l1_group(c, g):
                # 4 K=8 matmuls packed into the PE row groups (x/W1 are
                # host-replicated at partitions 0/32/64/96).
                key = ("h1", c)
                if key not in h_tiles:
                    h_tiles[key] = hp.tile(
                        [P, KT, NB], F8, tag="h1", name=f"h1_{c}", bufs=3
                    )
                h1_t = h_tiles[key]
                pss = []
                for i in range(4):
                    m = g * 4 + i
                    # chunk 0 runs unpacked on PE rows 0-7: it then depends
                    # only on the FIRST wx row-group DMA semaphore, so the
                    # PE starts ~1.3us earlier; the extra runtime fills the
                    # weight-wait window.
                    r0 = 0 if c == 0 else 32 * i
                    ps = pp.tile([P, NB], F32, tag="ps", name=f"ps1_{c}_{m}")
                    nc.tensor.matmul(
                        ps[:],
                        w1_sb[r0 : r0 + D1, m * P : (m + 1) * P],
                        wx_sb[r0 : r0 + D1, H + c * NB : H + (c + 1) * NB],
                        start=True,
                        stop=True,
                        tile_position=(r0, 0) if c > 0 else None,
                    )
                    pss.append(ps)
                for i in range(4):
                    relu_store(g * 4 + i, pss[i], h1_t, None, 0)

            def emit_l1(c):
                emit_l1_group(c, 0)
                emit_l1_group(c, 1)

            def emit_hidden(c, lname, h_in, h_out, w_sb, b_sb, boff, mid=None):
                for m in range(KT):
                    ps = pp.tile([P, NB], F32, tag="ps", name=f"ps{lname}_{c}_{m}")
                    for q in range(QT):
                        nc.tensor.matmul(
                            ps[:],
                            w_sb[:, 2 * q : 2 * q + 2, m * P : (m + 1) * P],
                            h_in[:, 2 * q : 2 * q + 2, :],
                            start=(q == 0),
                            stop=(q == QT - 1),
                            perf_mode=DR,
                        )
                    relu_store(m, ps, h_out, b_sb, boff)
                    if m == 3 and mid is not None:
                        mid()

            # ---- L1 software-pipelined two chunks ahead ----------------
            emit_l1(0)
            emit_l1(1)

            # ---- constraint bounds from x (independent of the MLP).
            # Emitted here so the DVE-queue work lands after chunk 0/1's h1
            # ReLUs but well before the first raw conversion needs `ub`.
            def col(c):
                return xc_sb[:, c * JT : (c + 1) * JT]

            cem, slag, fly, wat, ager = col(0), col(1), col(2), col(3), col(7)

            def ctile(name):
                return cp.tile([P, JT], F32, tag=name, name=name)

            def mtile(name):
                return cp.tile([P, JT], mybir.dt.uint8, tag=name, name=name)

            vec = nc.vector

            age = ctile("age")
            vec.tensor_single_scalar(age[:], ager, 1.0, ALU.max)
            cmask = mtile("cmask")
            vec.tensor_single_scalar(cmask[:], cem, 0.0, ALU.is_gt)
            wmask = mtile("wmask")
            vec.tensor_single_scalar(wmask[:], wat, 0.0, ALU.is_gt)
            vmask = mtile("vmask")
            vec.tensor_tensor(vmask[:], cmask[:], wmask[:], ALU.bitwise_and)
            ones = ctile("ones")
            vec.memset(ones[:], 1.0)
            cems = ctile("cems")
            vec.select(cems[:], cmask[:], cem, ones[:])
            rcem = ctile("rcem")
            vec.reciprocal(rcem[:], cems[:])
            wc = ctile("wc")
            vec.tensor_tensor(wc[:], wat, rcem[:], ALU.mult)
            scm = ctile("scm")
            vec.tensor_tensor(scm[:], slag, fly, ALU.add)
            binder = ctile("binder")
            vec.tensor_tensor(binder[:], cem, scm[:], ALU.add)
            den1 = ctile("den1")
            vec.tensor_single_scalar(den1[:], binder[:], 0.1, ALU.max)
            rden1 = ctile("rden1")
            vec.reciprocal(rden1[:], den1[:])
            r1s = ctile("r1s")
            vec.tensor_tensor(r1s[:], scm[:], rden1[:], ALU.mult)
            amax = ctile("amax")
            vec.tensor_scalar(amax[:], r1s[:], -0.15, 0.95, ALU.mult, ALU.add)
            hyd = ctile("hyd")
            vec.tensor_single_scalar(hyd[:], wc[:], 1.0, ALU.add)
            rhyd = ctile("rhyd")
            vec.reciprocal(rhyd[:], hyd[:])
            ea = ctile("ea")
            vec.tensor_tensor(ea[:], rhyd[:], age[:], ALU.mult)
            ex = ctile("ex")
            nc.scalar.activation(ex[:], ea[:], AF.Exp, scale=-0.01)
            omex = ctile("omex")
            vec.tensor_scalar(omex[:], ex[:], -1.0, 1.0, ALU.mult, ALU.add)
            alpha = ctile("alpha")
            vec.tensor_tensor(alpha[:], amax[:], omex[:], ALU.mult)
            bmask = mtile("bmask")
            vec.tensor_single_scalar(bmask[:], binder[:], 0.0, ALU.is_gt)
            bsafe = ctile("bsafe")
            vec.select(bsafe[:], bmask[:], binder[:], ones[:])
            rbs = ctile("rbs")
            vec.reciprocal(rbs[:], bsafe[:])
            cf = ctile("cf")
            vec.tensor_tensor(cf[:], cem, rbs[:], ALU.mult)
            acf = ctile("acf")
            vec.tensor_tensor(acf[:], alpha[:], cf[:], ALU.mult)
            wcmask = mtile("wcmask")
            vec.tensor_single_scalar(wcmask[:], wc[:], 0.0, ALU.is_gt)
            wcsafe = ctile("wcsafe")
            vec.select(wcsafe[:], wcmask[:], wc[:], ones[:])
            rwcs = ctile("rwcs")
            vec.reciprocal(rwcs[:], wcsafe[:])
            gel = ctile("gel")
            vec.tensor_tensor(gel[:], acf[:], rwcs[:], ALU.mult)
            g = ctile("g")
            vec.tensor_scalar(g[:], gel[:], 0.01, 10.0, ALU.max, ALU.min)
            g2 = ctile("g2")
            vec.tensor_tensor(g2[:], g[:], g[:], ALU.mult)
            g3 = ctile("g3")
            vec.tensor_tensor(g3[:], g2[:], g[:], ALU.mult)
            phys = ctile("phys")
            vec.tensor_scalar(phys[:], g3[:], 50.0, 5.0, ALU.mult, ALU.max)
            physl = ctile("physl")
            vec.tensor_single_scalar(physl[:], phys[:], 120.0, ALU.min)
            tot1 = ctile("tot1")
            vec.tensor_tensor(tot1[:], cem, wat, ALU.add)
            total = ctile("total")
            vec.tensor_tensor(total[:], tot1[:], scm[:], ALU.add)
            dtot = ctile("dtot")
            vec.tensor_single_scalar(dtot[:], total[:], 1e-6, ALU.max)
            rtot = ctile("rtot")
            vec.reciprocal(rtot[:], dtot[:])
            cfac = ctile("cfac")
            vec.tensor_tensor(cfac[:], cem, rtot[:], ALU.mult)
            cons = ctile("cons")
            vec.tensor_single_scalar(cons[:], cfac[:], 120.0, ALU.mult)
            ub = ctile("ub")
            vec.tensor_tensor(ub[:], physl[:], cons[:], ALU.min)
            amask = mtile("amask")
            vec.tensor_tensor(amask[:], vmask[:], bmask[:], ALU.bitwise_and)

            # ---- raw [1, w] -> batch-major clamp -> store --------------
            raw_bm = cp.tile([P, JT], F32, tag="raw_bm")
            rawb = ctile("rawb")
            lo5 = ctile("lo5")
            constr = ctile("constr")
            outsb = cp.tile([P, JT], F32, tag="outsb")
            nj = NB // P  # batch-major columns produced per chunk

            def raw_to_out(c, ps_part, rows, part_id, eng=None):
                # batch-major mapping: sample n of chunk c lives at
                # (partition n//4, column c*4 + n%4).  `rows` selects the
                # partition range this piece covers (full or half chunk).
                eng = eng or nc.sync
                w = (rows.stop - rows.start) * nj
                rawt = rp.tile([1, w], F32, tag="rawt", name=f"rawt{c}_{part_id}")
                vec.tensor_copy(rawt[:], ps_part)
                sl = slice(c * nj, (c + 1) * nj)
                # single strided SBUF->SBUF DMA does the [1, w] -> [p, 4]
                # batch-major transpose in one hop.
                eng.dma_start(
                    raw_bm[rows, sl],
                    rawt[0:1, :].rearrange("o (p j) -> o p j", j=nj),
                )
                vec.tensor_scalar(
                    rawb[rows, sl], raw_bm[rows, sl], 1.0 / RAW_DIV,
                    b4_sb[rows, 0:1], ALU.mult, ALU.add,
                )
                vec.tensor_single_scalar(lo5[rows, sl], rawb[rows, sl], 5.0, ALU.max)
                vec.tensor_tensor(
                    constr[rows, sl], lo5[rows, sl], ub[rows, sl], ALU.min
                )
                vec.select(
                    outsb[rows, sl], amask[rows, sl], constr[rows, sl],
                    rawb[rows, sl],
                )
                nc.gpsimd.dma_start(out_d[rows, sl], outsb[rows, sl])

            def raw_tr_out(c, ps_part, j0, part_id):
                # last-chunk path: PE-transpose [1, 256] -> two PSUM columns
                # (no DMA, no completion-semaphore latency in the tail).
                # mapping for this chunk: sample n -> (p = n % 128,
                # column c*4 + n // 128).
                w = 2 * P
                rawt = rp.tile([1, w], F32, tag="rawt", name=f"rawtT{c}_{part_id}")
                nc.scalar.copy(rawt[:], ps_part)
                psT = pp.tile([P, 4], F32, tag="ps", name=f"psT{c}_{part_id}")
                for jj in range(2):
                    nc.tensor.matmul(
                        psT[:, j0 + jj : j0 + jj + 1],
                        rawt[0:1, jj * P : (jj + 1) * P],
                        ones[0:1, 0:1],
                        is_transpose=True,
                    )
                sl = slice(c * nj + j0, c * nj + j0 + 2)
                vec.tensor_scalar(
                    rawb[:, sl], psT[:, j0 : j0 + 2], 1.0 / RAW_DIV,
                    b4_sb[:, 0:1], ALU.mult, ALU.add,
                )
                vec.tensor_single_scalar(lo5[:, sl], rawb[:, sl], 5.0, ALU.max)
                vec.tensor_tensor(constr[:, sl], lo5[:, sl], ub[:, sl], ALU.min)
                vec.select(outsb[:, sl], amask[:, sl], constr[:, sl], rawb[:, sl])
                nc.sync.dma_start(out_d[:, sl], outsb[:, sl])

            # ---- main chunk loop ---------------------------------------
            for c in range(NCH):
                h1_t = h_tiles[("h1", c)]
                h2_t = hp.tile([P, KT, NB], F8, tag="h2", name=f"h2_{c}", bufs=3)
                # L1 lookahead groups ride inside/after L2's m-loop so their
                # PSUM slots reuse buffers whose ReLUs finished long ago.
                mid = (lambda: emit_l1_group(c + 2, 0)) if c + 2 < NCH else None
                emit_hidden(c, "2", h1_t, h2_t, w2_sb, b23_sb, 0, mid=mid)
                if c + 2 < NCH:
                    emit_l1_group(c + 2, 1)
                h3_t = hp.tile([P, KT, NB], F8, tag="h3", name=f"h3_{c}", bufs=3)
                emit_hidden(c, "3", h2_t, h3_t, w3_sb, b23_sb, KT)

                if c < NCH - 1:
                    ps4 = pp.tile([2, NB], F32, tag="ps", name=f"ps4_{c}")
                    for q in range(QT):
                        nc.tensor.matmul(
                            ps4[:],
                            w4_sb[:, 2 * q : 2 * q + 2, 0:2],
                            h3_t[:, 2 * q : 2 * q + 2, :],
                            start=(q == 0),
                            stop=(q == QT - 1),
                            perf_mode=DR,
                        )
                    raw_to_out(c, ps4[0:1, :], slice(0, P), "a")
                else:
                    # last chunk: split L4 into halves so the first half's
                    # raw conversion overlaps the second half's matmuls.
                    HB = NB // 2
                    ps4a = pp.tile([2, HB], F32, tag="ps", name="ps4_la")
                    ps4b = pp.tile([2, HB], F32, tag="ps", name="ps4_lb")
                    for q in range(QT):
                        nc.tensor.matmul(
                            ps4a[:],
                            w4_sb[:, 2 * q : 2 * q + 2, 0:2],
                            h3_t[:, 2 * q : 2 * q + 2, :HB],
                            start=(q == 0),
                            stop=(q == QT - 1),
                            perf_mode=DR,
                        )
                    # second half's matmuls run while ACT copies the first
                    # half out of PSUM, so the PE transposes don't stall.
                    for q in range(QT):
                        nc.tensor.matmul(
                            ps4b[:],
                            w4_sb[:, 2 * q : 2 * q + 2, 0:2],
                            h3_t[:, 2 * q : 2 * q + 2, HB:],
                            start=(q == 0),
                            stop=(q == QT - 1),
                            perf_mode=DR,
                        )
                    raw_tr_out(c, ps4a[0:1, :], 0, "a")
                    raw_tr_out(c, ps4b[0:1, :], 2, "b")

    nc.compile()
    return nc


def _get_nc():
    if "nc" not in _CACHE:
        _CACHE["nc"] = _build_nc()
    return _CACHE["nc"]


def _prep_in_maps(x, W1, b1, W2, b2, W3, b3, W4, b4):
    f = np.float32
    f8 = mybir.dt.np(F8)
    x = np.ascontiguousarray(np.asarray(x, f))
    W1 = np.asarray(W1, f)
    W2 = np.asarray(W2, f)
    W3 = np.asarray(W3, f)
    W4 = np.asarray(W4, f)
    # [p, k, m] = W[k*128+p, m], scaled + quantized to fp8
    w2p = np.ascontiguousarray(
        (S2 * W2).reshape(KT, P, H).transpose(1, 0, 2).astype(f8)
    )
    w3p = np.ascontiguousarray(
        (S3 * W3).reshape(KT, P, H).transpose(1, 0, 2).astype(f8)
    )
    w4p = np.zeros((P, KT, 16), f8)
    w4c = (S4 * W4).reshape(KT, P).T.astype(f8)
    w4p[:, :, 0] = w4c
    w4p[:, :, 1] = w4c
    b1p = (S1 * np.asarray(b1, f)).reshape(KT, P).T
    b2p = (S1 * S2 * np.asarray(b2, f)).reshape(KT, P).T
    b3p = (S1 * S2 * S3 * np.asarray(b3, f)).reshape(KT, P).T
    b4p = np.full((P, 1), np.asarray(b4, f).reshape(-1)[0], f)
    bconsts_c = np.ascontiguousarray(np.concatenate([b1p, b2p, b3p, b4p], axis=1))

    in_maps = []
    for c in range(N_CORES):
        sl = x[c * BC : (c + 1) * BC]  # [4096, 8]
        # wx = [w1 | x^T] in bf16, host-replicated 4x for the PE row
        # groups.  bf16 x only feeds the MLP (clamp-protected); the
        # constraint path uses exact fp32 x via xc.
        bf = mybir.dt.np(BF16)
        wx1 = np.empty((D_IN + 1, H + BC), f)
        wx1[:D_IN, :H] = S1 * W1
        wx1[:D_IN, H:] = sl.T
        wx1[D_IN, :H] = S1 * np.asarray(b1, f)
        wx1[D_IN, H:] = 1.0
        wx_c = np.tile(wx1.astype(bf), (4, 1))
        # chunks 0-6: xc[p, f*JT + c*4 + j] = sl[c*512 + p*4 + j, f]
        # chunk 7 (PE-transpose path): xc[p, f*JT + 28 + j] = sl[3584 + j*128 + p, f]
        xc_c = (
            sl.reshape(NCH, P, NB // P, D_IN)
            .transpose(1, 3, 0, 2)
            .reshape(P, D_IN * JT)
            .copy()
        )
        last = sl[7 * NB :].reshape(NB // P, P, D_IN)  # [j, p, f]
        xcv = xc_c.reshape(P, D_IN, JT)
        xcv[:, :, 7 * (NB // P) :] = last.transpose(1, 2, 0)
        xc_c = np.ascontiguousarray(xc_c)
        in_maps.append(
            {
                "wx": np.ascontiguousarray(wx_c),
                "bconsts": bconsts_c,
                "xc": xc_c,
                "w2": w2p,
                "w3": w3p,
                "w4": w4p,
            }
        )
    return in_maps


def kernel(x, W1, b1, W2, b2, W3, b3, W4, b4, **run_kwargs):
    nc = _get_nc()
    in_maps = _prep_in_maps(x, W1, b1, W2, b2, W3, b3, W4, b4)
    res = run_bass_kernel_spmd(nc, in_maps, core_ids=list(range(N_CORES)), **run_kwargs)
    out = np.empty((B, 1), np.float32)
    for c in range(N_CORES):
        obm = res.results[c]["out_bm"].reshape(P, NCH, NB // P)
        core = obm.transpose(1, 0, 2).reshape(BC).copy()
        core[7 * NB :] = obm[:, 7, :].T.reshape(NB)
        out[c * BC : (c + 1) * BC, 0] = core
    if run_kwargs:
        kernel.last_results = res
    return out
